# revision 15
# baseline (speedup 1.0000x reference)
"""Causal single-head attention on 8 trn2 NeuronCores.

Problem: x[4,4096,768], WQ/WK/WV[768,64] -> out[4,4096,64]
  Q=x@WQ K=x@WK V=x@WV; causal softmax(QK^T/8)@V per batch.

Sharding: 2 cores per batch. Causal-balanced query split: program A
handles q-blocks {0,3,4,7} (of 512 rows), program B {1,2,5,6} -- both
have exactly 72 visible (k-chunk, q-block) pairs. The two SPMD programs
run concurrently on devices [0:4] and [4:8] (async jax dispatch).

Design (122us v1 baseline -> 88 -> 67 -> 64 -> ~59us; trace-driven,
see the NTFF recipe in profile_run.py):
  - x is staged HOST-side transposed AND cast to bf16 (xt[768,4096]).
    One HWDGE DMA per 512-row s-block lands [128, 6, 512] bf16 tiles
    directly: deletes all 192 PE x-transposes (fp32, 2 cyc/col) plus
    their PSUM banks and DVE copies that dominated v1, and halves x
    HBM traffic. (SWDGE cast-DMA f32->bf16 works single-shot but hits
    a walrus "ISA wrong length" codegen bug inside For_i; bf16
    stationary x f32r moving is NCC_IBIR034-illegal, hence host cast.)
  - Whole PE pipeline in bf16 (psum accumulation stays fp32): enables
    Fast Weight Load (LDWEIGHTS 219 -> ~131ns measured) and 1 cyc/col
    at any moving width.
  - Row-packed score matmuls: contraction is DK=64, so two score
    matmuls run CONCURRENTLY in the PE array (64x128 tiling mode,
    tile_position auto-derived from operand partitions; verified
    row_grp=h0/h64 with ~35ns stagger in the HW trace). KT is placed
    top (partitions 0:64) for even s-blocks via the [WK|WV] stationary
    and bottom (64:128) for odd s-blocks via [WV|WK]; QT is produced
    duplicated on both halves for free via a [WQ|WQ] stationary. Packs
    pair a chunk of an even block with one of the following odd block.
  - Batched exp: each pack's two [128,512] score tiles live in one
    2-bank PSUM tile [128,2,512]; ONE activation instruction exps all
    1024 columns. ACT is 1 elem/lane/cycle @1.2GHz dtype-independent
    with ~440cyc fixed cost, so halving the instruction count matters
    (36 exps ~= 40us ACT busy, the hard floor of this problem).
  - pt/VP bf16; PV accumulates fp32 in PSUM; the softmax denominator
    rides along as VP's ones column (row 64 of OT); PV column-trimmed
    on diagonal chunks; causal mask via affine_select on just the
    128-wide triangle slice.
  - The first two steps' x tiles are preloaded at the TAIL of each
    loop body (and once before the loop): the For_i back-edge is a
    full all-engine barrier, so without this every iteration starts
    with a serial x-DMA wait that leaves the PE idle long enough for
    the HAM clock-gate to re-throttle it to 1.2GHz.
  - PSUM: st 2x2 banks + ot 2 + kvqt 2 = 8; epilogue normalization on
    DVE (reciprocal * mul), output via PE transpose of [O|denom].

BENCH_LOOP_REPS=N wraps the whole per-forward body in a hardware For_i
loop (N iterations back-to-back on device, unroll 8 per back-edge) so
steady-state per-forward time can be measured past the ~8-12ms axon
per-execute overhead.
"""
import sys
import os

sys.path.insert(0, "/opt/trn_rl_repo")

import numpy as np

B, S, DM, DK = 4, 4096, 768, 64
NSB = S // 512  # 8 s-blocks of 512 rows
EVEN_BLOCKS = [0, 3, 4, 7]  # program A q-blocks
ODD_BLOCKS = [1, 2, 5, 6]  # program B q-blocks

_cache = {}


def _split_waits(nc, mybir, maxw=1):
    """Walrus here accepts only 1 sem-wait per instruction; move excess
    waits onto preceding same-engine no-ops."""
    cnt = 0
    for bb in nc.m.functions[0].blocks:
        new_insts = []
        for inst in bb.instructions:
            si = inst.sync_info
            if si is not None and si.on_wait and len(si.on_wait) > maxw:
                waits = list(si.on_wait)
                si.on_wait = waits[:maxw]
                extra = waits[maxw:]
                for i in range(0, len(extra), maxw):
                    cnt += 1
                    nop = mybir.InstNoOp(name=f"waitsplit-{cnt}", ins=[], outs=[])
                    nop.engine = inst.engine
                    nop.sync_info = mybir.SyncInfo(
                        on_wait=extra[i : i + maxw], on_update=[]
                    )
                    new_insts.append(nop)
            new_insts.append(inst)
        bb.instructions[:] = new_insts
    return nc


def _schedule(blocks):
    """Step order: own-q-block prefetches (Q<J>) interleaved into the
    K/V stream (S<sb>) so q-projections are ready early and attention
    packs spread across the stream. Returns list of ('S'|'Q', idx)."""
    if blocks == [0, 3, 4, 7]:
        return [("S", 0), ("Q", 3), ("S", 1), ("Q", 4), ("S", 2), ("S", 3),
                ("Q", 7), ("S", 4), ("S", 5), ("S", 6), ("S", 7)]
    if blocks == [1, 2, 5, 6]:
        return [("S", 0), ("S", 1), ("S", 2), ("Q", 5), ("S", 3), ("Q", 6),
                ("S", 4), ("S", 5), ("S", 6)]
    raise ValueError(blocks)


def _build_program(blocks, split_waits=True):
    import concourse.bass as bass
    import concourse.mybir as mybir
    from concourse.tile import TileContext
    from concourse.masks import make_identity

    f32 = mybir.dt.float32
    f32r = mybir.dt.float32r
    bf16 = mybir.dt.bfloat16
    AF = mybir.ActivationFunctionType

    nc = bass.Bass()
    xt = nc.declare_dram_parameter("xt", [DM, S], bf16, isOutput=False)
    wq = nc.declare_dram_parameter("wq", [DM, DK], f32, isOutput=False)
    wk = nc.declare_dram_parameter("wk", [DM, DK], f32, isOutput=False)
    wv = nc.declare_dram_parameter("wv", [DM, DK], f32, isOutput=False)
    out = nc.declare_dram_parameter("out", [4 * 512, DK], f32, isOutput=True)
    xt_r = xt.rearrange("(c p) s -> p c s", p=128)

    loop_reps = int(os.environ.get("BENCH_LOOP_REPS", "1"))
    steps = _schedule(blocks)

    with TileContext(nc) as tc:
        with (
            tc.tile_pool(name="consts", bufs=1) as cpool,
            tc.tile_pool(name="big", bufs=1) as big,
        ):
            # weights: one coarse DMA each into fp32 staging, then DVE
            # copies build the packed bf16 stationaries (bf16 enables
            # Fast Weight Load -- halves LDWEIGHTS)
            wk_s = cpool.tile([128, 6, 64], f32)
            wv_s = cpool.tile([128, 6, 64], f32)
            wq_s = cpool.tile([128, 6, 64], f32)
            nc.sync.dma_start(wk_s[:], wk.rearrange("(c p) d -> p c d", p=128))
            nc.sync.dma_start(wv_s[:], wv.rearrange("(c p) d -> p c d", p=128))
            nc.sync.dma_start(wq_s[:], wq.rearrange("(c p) d -> p c d", p=128))
            ident = cpool.tile([128, 128], f32)
            make_identity(nc, ident[:])
            ident_r = cpool.tile([128, 128], f32r)
            nc.vector.tensor_copy(ident_r[:], ident[:])
            ident_b = cpool.tile([128, 128], bf16)
            nc.vector.tensor_copy(ident_b[:], ident[:])
            # stationaries: [WK|WV] (KT on out rows 0:64 -- even blocks),
            # [WV|WK] (KT on rows 64:128 -- odd blocks), [WQ|WQ] (QT
            # duplicated on both halves for the row-packed score rhs)
            wkv_e = cpool.tile([128, 6 * 128], bf16)
            wkv_o = cpool.tile([128, 6 * 128], bf16)
            wqq = cpool.tile([128, 6 * 128], bf16)
            wkv_e_v = wkv_e[:].rearrange("p (c two d) -> p c two d", two=2, d=64)
            wkv_o_v = wkv_o[:].rearrange("p (c two d) -> p c two d", two=2, d=64)
            wqq_v = wqq[:].rearrange("p (c two d) -> p c two d", two=2, d=64)
            nc.vector.tensor_copy(wkv_e_v[:, :, 0, :], wk_s[:])
            nc.vector.tensor_copy(wkv_e_v[:, :, 1, :], wv_s[:])
            nc.vector.tensor_copy(wkv_o_v[:, :, 0, :], wv_s[:])
            nc.vector.tensor_copy(wkv_o_v[:, :, 1, :], wk_s[:])
            nc.vector.tensor_copy(wqq_v[:, :, 0, :], wq_s[:])
            nc.vector.tensor_copy(wqq_v[:, :, 1, :], wq_s[:])

            # KTVT: per s-block parity, rows 0:64|64:128 hold KT|VT
            # (even) or VT|KT (odd)
            KTVT = big.tile([128, S], bf16)
            QT = big.tile([128, 4 * 512], bf16)  # rows 64:128 duplicate
            VP = big.tile([128, 32 * 66], bf16)  # [V|1|0] per k-chunk
            # kept x^T for own q-blocks (loaded once, reused by stream)
            xtq = {J: big.tile([128, 6, 512], bf16, name=f"xtq{J}") for J in blocks}
            # prefetch tile for the first non-own stream block (B only)
            pre0 = None
            if steps[0][1] not in blocks:
                pre0 = big.tile([128, 6, 512], bf16, name="pre0")
            VPr = VP[:].rearrange("p (c u) -> p c u", u=66)
            ones_s = cpool.tile([128, 32], f32)
            zero_s = cpool.tile([128, 32], f32)
            nc.gpsimd.memset(ones_s[:], 1.0)
            nc.gpsimd.memset(zero_s[:], 0.0)
            nc.vector.tensor_copy(VPr[:, :, 64], ones_s[:])
            nc.vector.tensor_copy(VPr[:, :, 65], zero_s[:])

            # PSUM budget (8 banks): st 2x2 + ot 2 + kvqt 2
            with (
                tc.tile_pool(name="xload", bufs=3) as xl_pool,
                tc.tile_pool(name="kv_ps", bufs=2, space="PSUM") as kv_psum,
                tc.tile_pool(name="s_ps", bufs=2, space="PSUM") as s_psum,
                tc.tile_pool(name="pt", bufs=6) as pt_pool,
                tc.tile_pool(name="ot_ps", bufs=2, space="PSUM") as ot_psum,
                tc.tile_pool(name="ep", bufs=2) as ep_pool,
            ):
                state = {}

                def load_x(sb, xts):
                    """one DMA per s-block, 2KiB contiguous descriptors."""
                    nc.sync.dma_start(
                        xts[:], xt_r[:, :, sb * 512 : (sb + 1) * 512]
                    )

                def project_qt(qb, xts):
                    qt = kv_psum.tile([128, 512], f32, name="qt", tag="kvqt")
                    for c in range(6):
                        nc.tensor.matmul(
                            qt[:],
                            wqq[:, c * 128 : (c + 1) * 128],
                            xts[:, c, :],
                            start=(c == 0),
                            stop=(c == 5),
                        )
                    nc.vector.tensor_copy(QT[:, qb * 512 : (qb + 1) * 512], qt[:])

                def project_kv(sb, xts):
                    wkv = wkv_e if sb % 2 == 0 else wkv_o
                    kv = kv_psum.tile([128, 512], f32, name="kv", tag="kvqt")
                    for c in range(6):
                        nc.tensor.matmul(
                            kv[:],
                            wkv[:, c * 128 : (c + 1) * 128],
                            xts[:, c, :],
                            start=(c == 0),
                            stop=(c == 5),
                        )
                    nc.vector.tensor_copy(KTVT[:, sb * 512 : (sb + 1) * 512], kv[:])
                    vt_rows = slice(64, 128) if sb % 2 == 0 else slice(0, 64)
                    vtp = kv_psum.tile([128, 4, 64], bf16, name="vtp", tag="kvqt")
                    for u in range(4):
                        kc = sb * 4 + u
                        nc.tensor.transpose(
                            vtp[:, u, :],
                            KTVT[vt_rows, kc * 128 : (kc + 1) * 128],
                            ident_b[vt_rows, vt_rows],
                        )
                    nc.vector.tensor_copy(
                        VPr[:, sb * 4 : (sb + 1) * 4, 0:64], vtp[:]
                    )

                def score_mm(st_half, kc, qb, rows):
                    nc.tensor.matmul(
                        st_half,
                        KTVT[rows, kc * 128 : (kc + 1) * 128],
                        QT[rows, qb * 512 : (qb + 1) * 512],
                        start=True,
                        stop=True,
                    )

                def emit_pack(qb, J, kc_pair, diag_i):
                    """Two score chunks -> one 2-bank st tile -> one exp
                    -> masked -> two PV accumulations into OT[qb].
                    kc_pair = (kc_top, kc_bot); kc_bot None for a
                    pseudo-pack of two same-parity (diag) chunks packed
                    only for the shared exp. diag_i = (i_top, i_bot)
                    with i = kc-4J if diagonal else None."""
                    kc_t, kc_b = kc_pair
                    i_t, i_b = diag_i
                    st = s_psum.tile([128, 2, 512], f32, name="st", tag="st")
                    rows_t = slice(0, 64) if (kc_t // 4) % 2 == 0 else slice(64, 128)
                    rows_b = slice(0, 64) if (kc_b // 4) % 2 == 0 else slice(64, 128)
                    score_mm(st[:, 0, :], kc_t, qb, rows_t)
                    score_mm(st[:, 1, :], kc_b, qb, rows_b)
                    pt = pt_pool.tile([128, 2, 512], bf16, name="pt", tag="pt")
                    # both halves diagonal: exp only the union of the live
                    # windows (columns left of it are never read)
                    lo_e = 128 * min(i_t or 0, i_b or 0)
                    nc.scalar.activation(
                        pt[:, :, lo_e:], st[:, :, lo_e:], AF.Exp, scale=0.125
                    )
                    for half, (kc, i) in enumerate(((kc_t, i_t), (kc_b, i_b))):
                        if i is not None:
                            # triangle mask on the 128-wide diagonal slice
                            nc.gpsimd.affine_select(
                                out=pt[:, half, 128 * i : 128 * (i + 1)],
                                in_=pt[:, half, 128 * i : 128 * (i + 1)],
                                compare_op=mybir.AluOpType.is_ge,
                                fill=0.0,
                                base=0,
                                pattern=[[1, 128]],
                                channel_multiplier=-1,
                            )
                    for half, (kc, i) in enumerate(((kc_t, i_t), (kc_b, i_b))):
                        lo = 0 if i is None else 128 * i
                        npairs = state["count"][qb]
                        nc.tensor.matmul(
                            state["ot"][qb][:, lo:512],
                            VPr[:, kc, :],
                            pt[:, half, lo:512],
                            start=(npairs == 0),
                            stop=(npairs == 4 * J + 3),
                        )
                        state["count"][qb] += 1

                def emit_epilogue(qb):
                    ots = ep_pool.tile([66, 512], f32r, name="ots", tag="ots")
                    nc.vector.tensor_copy(ots[:], state["ot"][qb][:, :])
                    o_n = ep_pool.tile([128, 4, 65], f32, name="o_n", tag="on")
                    rec = ep_pool.tile([128, 4], f32, name="rec", tag="rc")
                    for u in range(4):
                        tp2 = s_psum.tile([128, 2, 512], f32r, name="tp2", tag="st")
                        nc.tensor.transpose(
                            tp2[:, 0, 0:66],
                            ots[:, u * 128 : (u + 1) * 128],
                            ident_r[0:66, 0:66],
                        )
                        nc.vector.tensor_copy(o_n[:, u, :], tp2[:, 0, 0:65])
                        nc.vector.reciprocal(rec[:, u : u + 1], o_n[:, u, 64:65])
                        nc.vector.tensor_scalar_mul(
                            o_n[:, u, 0:64], o_n[:, u, 0:64], rec[:, u : u + 1]
                        )
                    ov = out[qb * 512 : (qb + 1) * 512, :].rearrange(
                        "(u p) d -> p u d", p=128
                    )
                    nc.sync.dma_start(ov, o_n[:, :, 0:64])

                def emit_eligible():
                    """Emit every not-yet-emitted pack whose KV blocks and
                    QT are ready, in k order within each qb."""
                    for m in range(state["kv_hi"] // 2):
                        for qb in range(4):
                            J = blocks[qb]
                            if (
                                J not in state["qt_ready"]
                                or 2 * m + 1 > J
                                or (m, qb) in state["emitted"]
                            ):
                                continue
                            diag_b = (2 * m + 1 == J)
                            if diag_b and J == blocks[-1]:
                                # final pack of the last q-block: handled
                                # split (below) so its top half doesn't
                                # wait for the last stream block
                                continue
                            state["emitted"].add((m, qb))
                            if qb not in state["ot"]:
                                state["ot"][qb] = ot_psum.tile(
                                    [66, 512], f32, name=f"ot{qb}", tag="ot"
                                )
                            for u in range(4):
                                kc_t, kc_b = 8 * m + u, 8 * m + 4 + u
                                emit_pack(
                                    qb, J, (kc_t, kc_b),
                                    (None, (kc_b - 4 * J) if diag_b else None),
                                )
                            if state["count"][qb] == 4 * J + 4:
                                emit_epilogue(qb)
                    # split-emit the last q-block's final pack: its top
                    # half (block J-1, non-diag) goes one stream step
                    # earlier than the diagonal bottom half (block J), so
                    # the kernel tail only carries the diagonal chunks
                    J = blocks[-1]
                    if J % 2 == 1 and J in state["qt_ready"]:
                        qb = 3
                        if state["kv_hi"] >= J and ("T", qb) not in state["emitted"]:
                            state["emitted"].add(("T", qb))
                            if qb not in state["ot"]:
                                state["ot"][qb] = ot_psum.tile(
                                    [66, 512], f32, name=f"ot{qb}", tag="ot"
                                )
                            base = 4 * (J - 1)
                            emit_pack(qb, J, (base, base + 1), (None, None))
                            emit_pack(qb, J, (base + 2, base + 3), (None, None))
                        if (
                            state["kv_hi"] >= J + 1
                            and ("T", qb) in state["emitted"]
                            and ("B", qb) not in state["emitted"]
                        ):
                            state["emitted"].add(("B", qb))
                            emit_pack(qb, J, (4 * J, 4 * J + 1), (0, 1))
                            emit_pack(qb, J, (4 * J + 2, 4 * J + 3), (2, 3))
                            if state["count"][qb] == 4 * J + 4:
                                emit_epilogue(qb)
                    # unpaired diagonal block for even-J q-blocks: two
                    # pseudo-packs (chunks share an st tile + one exp but
                    # run sequentially on the PE top half)
                    for qb in range(4):
                        J = blocks[qb]
                        if (
                            J % 2 != 0
                            or J not in state["qt_ready"]
                            or state["kv_hi"] < J + 1
                            or ("D", qb) in state["emitted"]
                        ):
                            continue
                        state["emitted"].add(("D", qb))
                        if qb not in state["ot"]:
                            state["ot"][qb] = ot_psum.tile(
                                [66, 512], f32, name=f"ot{qb}", tag="ot"
                            )
                        for t in range(2):
                            i0, i1 = 2 * t, 2 * t + 1
                            emit_pack(
                                qb, J, (4 * J + i0, 4 * J + i1), (i0, i1)
                            )
                        if state["count"][qb] == 4 * J + 4:
                            emit_epilogue(qb)

                # the first two steps' x tiles are PRELOADED: before the
                # loop for iteration 0, and re-issued at the tail of each
                # body for the next iteration. This removes the serial
                # x-DMA wait after the For_i back-edge barrier that
                # otherwise leaves the PE idle (and HAM-throttled) at the
                # start of every iteration.
                pre_tiles = {}
                for kind, idx in steps[:2]:
                    if kind == "Q" or idx in blocks:
                        pre_tiles[idx] = xtq[idx]
                    else:
                        pre_tiles[idx] = pre0

                def preload():
                    for sb, t in pre_tiles.items():
                        load_x(sb, t)

                def body(_iv=None):
                    state.clear()
                    state.update(
                        emitted=set(),
                        qt_ready=set(),
                        kv_hi=0,
                        ot={},
                        count={qb: 0 for qb in range(4)},
                    )
                    for kind, idx in steps:
                        if kind == "Q":
                            J = idx
                            if J not in pre_tiles:
                                load_x(J, xtq[J])
                            project_qt(blocks.index(J), xtq[J])
                            state["qt_ready"].add(J)
                        else:
                            sb = idx
                            if sb in blocks:
                                xts = xtq[sb]
                                if sb not in state["qt_ready"]:
                                    if sb not in pre_tiles:
                                        load_x(sb, xts)
                                    project_qt(blocks.index(sb), xts)
                                    state["qt_ready"].add(sb)
                            elif sb in pre_tiles:
                                xts = pre_tiles[sb]
                            else:
                                xts = xl_pool.tile(
                                    [128, 6, 512], bf16, tag="xin"
                                )
                                load_x(sb, xts)
                            project_kv(sb, xts)
                            state["kv_hi"] = sb + 1
                        emit_eligible()
                    preload()  # stage next iteration's first x blocks

                preload()
                if loop_reps > 1:
                    # body is far larger than one IRAM block on every
                    # engine: arm back-edge branch prefetch hints
                    hints = (
                        mybir.EngineType.PE,
                        mybir.EngineType.DVE,
                        mybir.EngineType.Activation,
                        mybir.EngineType.Pool,
                        mybir.EngineType.SP,
                    )
                    tc.For_i_unrolled_general(
                        start=0,
                        end=loop_reps,
                        step=1,
                        unrollable_body=lambda iv0, unroll: [
                            body(iv0) for _ in range(unroll)
                        ],
                        max_unroll=8,
                        hint_engines=hints,
                    )
                else:
                    body()

    if split_waits:
        _split_waits(nc, mybir)
    return nc


def _make_runner(nc, n_cores, dev_offset):
    """Like bass2jax.run_bass_via_pjrt but with explicit device subset and
    reusable jitted callable."""
    import jax
    import concourse.mybir as mybir
    from concourse import bass2jax
    from jax.experimental.shard_map import shard_map
    from jax.sharding import Mesh, PartitionSpec

    bass2jax.install_neuronx_cc_hook()

    partition_name = (
        nc.partition_id_tensor.name if nc.partition_id_tensor else None
    )
    in_names, out_names, out_avals, zero_outs = [], [], [], []
    for alloc in nc.m.functions[0].allocations:
        if not isinstance(alloc, mybir.MemoryLocationSet):
            continue
        name = alloc.memorylocations[0].name
        if alloc.kind == "ExternalInput":
            if name != partition_name:
                in_names.append(name)
        elif alloc.kind == "ExternalOutput":
            shape = tuple(alloc.tensor_shape)
            dtype = mybir.dt.np(alloc.dtype)
            out_avals.append(jax.core.ShapedArray(shape, dtype))
            out_names.append(name)
            zero_outs.append(np.zeros(shape, dtype))
    n_params = len(in_names)
    n_outs = len(out_avals)
    all_names = in_names + out_names
    if partition_name is not None:
        all_names.append(partition_name)

    def _body(*args):
        operands = list(args)
        if partition_name is not None:
            operands.append(bass2jax.partition_id_tensor())
        outs = bass2jax._bass_exec_p.bind(
            *operands,
            out_avals=tuple(out_avals),
            in_names=tuple(all_names),
            out_names=tuple(out_names),
            lowering_input_output_aliases=(),
            sim_require_finite=True,
            sim_require_nnan=True,
            nc=nc,
        )
        return tuple(outs)

    devices = jax.devices()[dev_offset : dev_offset + n_cores]
    mesh = Mesh(np.asarray(devices), ("core",))
    in_specs = (PartitionSpec("core"),) * (n_params + n_outs)
    out_specs = (PartitionSpec("core"),) * n_outs
    sharded = jax.jit(
        shard_map(
            _body, mesh=mesh, in_specs=in_specs, out_specs=out_specs, check_rep=False
        ),
        keep_unused=True,
    )
    from jax.sharding import NamedSharding

    sh = NamedSharding(mesh, PartitionSpec("core"))

    def prepare(in_maps):
        per_core = [[np.asarray(m[n]) for n in in_names] for m in in_maps]
        concat_in = [
            np.concatenate([per_core[c][i] for c in range(n_cores)], axis=0)
            for i in range(n_params)
        ]
        concat_zeros = [
            np.zeros((n_cores * z.shape[0], *z.shape[1:]), z.dtype)
            for z in zero_outs
        ]
        return [jax.device_put(a, sh) for a in concat_in + concat_zeros]

    def run(in_maps):
        return sharded(*prepare(in_maps))

    run.sharded = sharded
    run.prepare = prepare
    run.out_names = out_names
    run.out_avals = out_avals
    run.n_cores = n_cores
    return run


def _get_runners():
    if "runners" not in _cache:
        nc_even = _build_program(EVEN_BLOCKS)
        nc_odd = _build_program(ODD_BLOCKS)
        _cache["runners"] = (
            _make_runner(nc_even, 4, 0),
            _make_runner(nc_odd, 4, 4),
        )
    return _cache["runners"]


def make_maps(x, WQ, WK, WV):
    import ml_dtypes

    return [
        {
            "xt": np.ascontiguousarray(np.asarray(x[b]).T).astype(
                ml_dtypes.bfloat16
            ),
            "wq": WQ,
            "wk": WK,
            "wv": WV,
        }
        for b in range(B)
    ]


def kernel(x, WQ, WK, WV):
    import jax

    run_even, run_odd = _get_runners()
    maps = make_maps(x, WQ, WK, WV)
    # async dispatch: program A on devices 0-3, program B on 4-7, concurrent
    oa = run_even(maps)
    ob = run_odd(maps)
    ra = np.asarray(oa[0]).reshape(4, 2048, DK)
    rb = np.asarray(ob[0]).reshape(4, 2048, DK)
    out = np.empty((B, S, DK), np.float32)
    for b in range(B):
        for i, J in enumerate(EVEN_BLOCKS):
            out[b, J * 512 : (J + 1) * 512] = ra[b, i * 512 : (i + 1) * 512]
        for i, J in enumerate(ODD_BLOCKS):
            out[b, J * 512 : (J + 1) * 512] = rb[b, i * 512 : (i + 1) * 512]
    return out


if __name__ == "__main__":
    rng = np.random.default_rng(0)
    x = rng.standard_normal((B, S, DM), dtype=np.float32)
    sc = 1.0 / np.sqrt(DM)
    WQ = rng.standard_normal((DM, DK), dtype=np.float32) * sc
    WK = rng.standard_normal((DM, DK), dtype=np.float32) * sc
    WV = rng.standard_normal((DM, DK), dtype=np.float32) * sc
    got = kernel(x, WQ, WK, WV)
    # numpy reference
    Q = x @ WQ
    K = x @ WK
    V = x @ WV
    sref = np.einsum("bqd,bkd->bqk", Q, K) / 8.0
    mask = np.tril(np.ones((S, S), bool))
    sref = np.where(mask, sref, -np.inf)
    sref = sref - sref.max(-1, keepdims=True)
    p = np.exp(sref)
    p /= p.sum(-1, keepdims=True)
    ref = np.einsum("bqk,bkv->bqv", p, V)
    err = np.abs(got - ref).max() / np.abs(ref).max()
    print("rel err:", err)
